# revision 1
# baseline (speedup 1.0000x reference)
"""DGINConv (2-layer GIN with edge features) Trainium2 kernel.

Math (per layer, reference):
    h_node = x @ W_node.T
    h_edge[i,j,:] = W_edge @ edges[i,j,:]
    ne = relu(h_node[None,:,:] + h_edge + bne)             # [N, N, din]
    msg = einsum('ij,ijd->id', adj, ne)
    out = relu(((1+eps)*x + msg) @ Wn.T + bn)

Algebraic restructure (requires adj in {0,1}):
    adj*relu(hb + he) = relu(hb + adj*he) - relu(hb) + adj*relu(hb)
With me := adj-masked edges (he is linear in edges):
    msg[i,d] = sum_j relu(hb[j,d] + mhe[i,j,d])      (term1, the big one)
             - sum_j relu(hb[j,d])                   (term2: per-layer const)
             + (adj @ relu(hb))[i,d]                 (term3: one small matmul)
The hb representation error cancels exactly between term1/term2 for
inactive pairs (masked-edge columns are exactly zero in PSUM).

Distribution: destination rows sharded 8 ways; nodes/weights replicated;
one AllGather of updated node features between layers.

term1 dataflow per own-row i (PSUM-exit bound — both vector engines split
the work):  PSUM[i] [128 d, 1024 j] (2 banks)
  = 4x row-tiled K=32 matmul of transposed masked edges (PE, tile_position)
  (+ Identity-matmul of hbT for ACT-consumed rows only)
  -> ACT rows: activation(Relu, accum_out)      = sum_j relu(hb + mhe)
  -> DVE rows: custom DVE op relu(in0+in1) with accumulate (hb added from
     SBUF by the op itself, no Identity-matmul needed).

me layout [128p=(t,k), g, jt, j]: xbar DMA-transpose of the natural-layout
bf16 masked edges; row-group 32t holds edges^T (k-major) of own row 4g+t,
feeding the 4 concurrent row-tiled matmuls directly.
"""

import sys

if "/opt/trn_rl_repo" not in sys.path:
    sys.path.insert(0, "/opt/trn_rl_repo")

import numpy as np

N, D, E, NC = 1024, 128, 32, 8
SH = N // NC          # 128 rows per core
NG = SH // 4          # 32 groups of 4 own-rows
NJT = N // 128        # 8 j-tiles
JH = 512
ACT_N32 = 16          # of every 32 groups, this many consumed by ACT
_cache = {}

_CUSTOM = {}


def _ensure_custom_op():
    """Register RELU_ADD_REDUCE_GIN: out = relu(in0 + in1); accum = sum."""
    if "op" in _CUSTOM:
        return _CUSTOM["op"]
    import concourse.dve_ops as dve_ops
    from concourse.dve_spec import Spec, Src0, Src1, relu, lower, _has_src1
    from concourse.dve_uop import DveOpSpec
    from operator import add

    name = "RELU_ADD_REDUCE_GIN"

    def _ref(in0, in1, c0, c1, c2):
        b = dve_ops._dve_relu(in0.astype(np.float32) + in1.astype(np.float32))
        return b, b.reshape(b.shape[0], -1).sum(axis=-1, keepdims=True)

    from concourse.dve_spec import Zero

    spec = Spec(body=relu(Src0 + Src1), accum=add, accum_init=Zero,
                reference=_ref)
    row = dve_ops._CUSTOM_DVE_ROW_BASE + len(dve_ops.OPS)
    assert row < 0x20
    shas = {}
    for ver in ("v3", "v4"):
        try:
            s = DveOpSpec(name=name, opcode=row, uops=lower(spec, ver=ver),
                          rd1_en=_has_src1(spec))
            shas[ver] = s.sha(ver)
        except Exception:
            pass
    op = dve_ops.DveOp(name, spec, subdim=False, uops_sha=shas)
    dve_ops.OPS.append(op)
    dve_ops.CUSTOM_DVE_SPECS[name] = spec
    dve_ops._SUB_OPCODE_FOR_NAME[name] = row
    _CUSTOM["op"] = op
    return op


def _build_nc(mode="full"):
    from contextlib import ExitStack

    import concourse.mybir as mybir
    import concourse.tile as tile
    from concourse import bacc
    from concourse.masks import make_identity

    relu_add_reduce = _ensure_custom_op()

    f32 = mybir.dt.float32
    bf16 = mybir.dt.bfloat16
    RELU = mybir.ActivationFunctionType.Relu
    ADD = mybir.AluOpType.add
    MULT = mybir.AluOpType.mult
    SUB = mybir.AluOpType.subtract
    MAX = mybir.AluOpType.max

    nc = bacc.Bacc("TRN2", target_bir_lowering=False, debug=False,
                   enable_asserts=False, num_devices=NC)

    def din(name, shape, dt=None):
        return nc.dram_tensor(name, shape, dt or f32, kind="ExternalInput").ap()

    edges_d = din("edges_sh", [SH, N, E], bf16)   # host pre-cast
    adjT_d = din("adjT_sh", [N, SH], bf16)        # host: adj[own].T, pre-cast
    xT_d = din("xT", [D, N], bf16)                # host: nodes.T, pre-cast
    xownT_d = din("x_ownT", [D, SH])        # host: nodes[own].T
    opse_d = din("opse", [128, 1])          # host: 1+eps replicated
    WeT_d = [din(f"WeT{l}", [E, D]) for l in range(2)]
    WnodeT_d = [din(f"WnodeT{l}", [D, D]) for l in range(2)]
    WnT_d = [din(f"WnT{l}", [D, D]) for l in range(2)]
    bne_d = [din(f"bne{l}", [D, 1]) for l in range(2)]
    bn_d = [din(f"bn{l}", [D, 1]) for l in range(2)]
    out_d = nc.dram_tensor("out", [SH, D], f32, kind="ExternalOutput").ap()

    with tile.TileContext(nc) as tc, ExitStack() as ctx:
        P = ctx.enter_context(tc.tile_pool(name="persist", bufs=1))
        dramp = ctx.enter_context(tc.tile_pool(name="dram", bufs=1, space="DRAM"))
        natp = ctx.enter_context(tc.tile_pool(name="nat", bufs=4))
        psumA = ctx.enter_context(tc.tile_pool(name="psumA", bufs=2, space="PSUM"))
        psumD = ctx.enter_context(tc.tile_pool(name="psumD", bufs=2, space="PSUM"))
        scrp = ctx.enter_context(tc.tile_pool(name="scr", bufs=3))
        scrd = ctx.enter_context(tc.tile_pool(name="scrd", bufs=3))

        # ---------------- constants / small inputs ----------------
        ident = P.tile([128, 128], bf16)
        make_identity(nc, ident[:])

        opse = P.tile([128, 1], f32)
        nc.sync.dma_start(out=opse[:], in_=opse_d[:])
        xownT = P.tile([D, SH], f32)
        nc.sync.dma_start(out=xownT[:], in_=xownT_d[:])

        WeT_rep, WnodeT, WnT, bne, bn = [], [], [], [], []
        for l in range(2):
            w = P.tile([128, 128], bf16, tag=f"WeTrep{l}")
            for t in range(4):
                nc.gpsimd.dma_start(out=w[32 * t:32 * (t + 1), :], in_=WeT_d[l][:])
            WeT_rep.append(w)
            wn = P.tile([D, D], bf16, tag=f"WnodeT{l}")
            nc.gpsimd.dma_start(out=wn[:], in_=WnodeT_d[l][:])
            WnodeT.append(wn)
            wo = P.tile([D, D], bf16, tag=f"WnT{l}")
            nc.gpsimd.dma_start(out=wo[:], in_=WnT_d[l][:])
            WnT.append(wo)
            b0 = P.tile([D, 1], f32, tag=f"bne{l}")
            nc.sync.dma_start(out=b0[:], in_=bne_d[l][:])
            bne.append(b0)
            b1 = P.tile([D, 1], f32, tag=f"bn{l}")
            nc.sync.dma_start(out=b1[:], in_=bn_d[l][:])
            bn.append(b1)

        # adjT: [jp, jt, i] bf16 (cast on DMA; host already transposed)
        adjT = P.tile([128, NJT, SH], bf16)
        nc.sync.dma_start(
            out=adjT[:], in_=adjT_d.rearrange("(jt p) i -> p jt i", p=128))

        # xT layer 0: [c, j] bf16
        xT0 = P.tile([D, N], bf16)
        nc.sync.dma_start(out=xT0[:], in_=xT_d[:])

        # ------------- edges: load(f32) + mask*cast + transpose -------------
        me = P.tile([128, NG, NJT, 128], bf16)
        for g in range(NG):
            natf = natp.tile([128, NJT, 4, E], bf16, tag="natf")
            for t in range(4):
                nc.sync.dma_start(
                    out=natf[:, :, t, :],
                    in_=edges_d[4 * g + t].rearrange("(jt p) k -> p jt k", p=128))
            natb = natp.tile([128, NJT, 4, E], bf16, tag="natb")
            adj_bc = adjT[:, :, 4 * g:4 * (g + 1)].unsqueeze(3).broadcast_to(
                [128, NJT, 4, E])
            meng = nc.gpsimd if (g % 2) == 0 else nc.vector
            meng.tensor_tensor(out=natb[:], in0=natf[:], in1=adj_bc, op=MULT)
            # batched xbar transpose: out[f,(jt),p] = in[p,(jt),f]
            nc.sync.dma_start(out=me[:, g, :, :], in_=natb[:], transpose=True)

        # ---------------- per-layer helpers ----------------
        def hb_prep(l, xT_l):
            hbT = P.tile([D, N], bf16, tag=f"hbT{l}")
            for h in range(2):
                ps = psumA.tile([128, JH], f32, tag="actps")
                nc.tensor.matmul(out=ps[:], lhsT=WnodeT[l][:],
                                 rhs=xT_l[:, h * JH:(h + 1) * JH],
                                 start=True, stop=True)
                nc.vector.tensor_scalar(out=hbT[:, h * JH:(h + 1) * JH],
                                        in0=ps[:], scalar1=bne[l][:],
                                        scalar2=None, op0=ADD)
            srh_p = P.tile([D, 2], f32, tag=f"srhp{l}")
            for h in range(2):
                scr = scrp.tile([128, 2 * JH], bf16, tag="scr_act")
                nc.scalar.activation(out=scr[:, 0:JH],
                                     in_=hbT[:, h * JH:(h + 1) * JH],
                                     func=RELU, accum_out=srh_p[:, h:h + 1])
            srh = P.tile([D, 1], f32, tag=f"srh{l}")
            nc.vector.tensor_tensor(out=srh[:], in0=srh_p[:, 0:1],
                                    in1=srh_p[:, 1:2], op=ADD)
            rhb_raw = P.tile([128, NJT, D], bf16, tag=f"rhbraw{l}")
            nc.sync.dma_start(out=rhb_raw[:], in_=hbT[:], transpose=True)
            reluhb = P.tile([128, NJT, D], bf16, tag=f"reluhb{l}")
            nc.vector.tensor_scalar(out=reluhb[:], in0=rhb_raw[:],
                                    scalar1=0.0, scalar2=None, op0=MAX)
            return hbT, srh, reluhb

        def layer(l, xT_l, xownT_l):
            hbT, srh, reluhb = hb_prep(l, xT_l)

            # term1: separate accumulators per consumer engine (avoids
            # cross-engine WAW serialization on a shared tile)
            msgA = P.tile([D, SH], f32, tag=f"msgA{l}")
            msgD = P.tile([D, SH], f32, tag=f"msgD{l}")
            nc.gpsimd.memset(msgA[:], 0.0)
            nc.gpsimd.memset(msgD[:], 0.0)
            for g in range(NG):
                use_act = ((g + 1) * ACT_N32 // NG) > (g * ACT_N32 // NG)
                pss = []
                for t in range(4):
                    if use_act:
                        ps = psumA.tile([128, 2 * JH], f32, tag="actps")
                    else:
                        ps = psumD.tile([128, 2 * JH], f32, tag="dveps")
                    pss.append(ps)
                if use_act:
                    for t in range(4):
                        for h in range(2):
                            nc.tensor.matmul(
                                out=pss[t][:, h * JH:(h + 1) * JH],
                                lhsT=ident[:],
                                rhs=hbT[:, h * JH:(h + 1) * JH],
                                start=True, stop=False)
                for t in range(4):
                    for h in range(2):
                        nc.tensor.matmul(
                            out=pss[t][:, h * JH:(h + 1) * JH],
                            lhsT=WeT_rep[l][32 * t:32 * (t + 1), :],
                            rhs=me[32 * t:32 * (t + 1), g, 4 * h:4 * (h + 1), :],
                            start=not use_act, stop=True,
                            tile_position=(32 * t, 0))
                for t in range(4):
                    idx = 4 * g + t
                    if use_act:
                        scr = scrp.tile([128, 2 * JH], bf16, tag="scr_act")
                        nc.scalar.activation(
                            out=scr[:], in_=pss[t][:], func=RELU,
                            accum_out=msgA[:, idx:idx + 1])
                    else:
                        scr = scrd.tile([128, 2 * JH], bf16, tag="scr_dve")
                        nc.vector._custom_dve(
                            relu_add_reduce, out=scr[:], in0=pss[t][:],
                            in1=hbT[:], accum_out=msgD[:, idx:idx + 1])

            # term3: corr[d, i] = sum_j reluhb[j, d] * adjT[j, i]
            ps_corr = psumA.tile([D, SH], f32, tag="actps")
            for jt in range(NJT):
                nc.tensor.matmul(out=ps_corr[:], lhsT=reluhb[:, jt, :],
                                 rhs=adjT[:, jt, :],
                                 start=(jt == 0), stop=(jt == NJT - 1))

            # z = (1+eps)*x_own + msg   (all [d, i_own])
            zm = P.tile([D, SH], f32, tag=f"zm{l}")
            nc.vector.scalar_tensor_tensor(out=zm[:], in0=msgA[:],
                                           scalar=srh[:], in1=msgD[:],
                                           op0=SUB, op1=ADD)
            zt = P.tile([D, SH], f32, tag=f"zt{l}")
            nc.vector.tensor_tensor(out=zt[:], in0=zm[:], in1=ps_corr[:],
                                    op=ADD)
            z_bf = P.tile([D, SH], bf16, tag=f"zbf{l}")
            nc.vector.scalar_tensor_tensor(out=z_bf[:], in0=xownT_l[:],
                                           scalar=opse[:], in1=zt[:],
                                           op0=MULT, op1=ADD)

            # h = relu(Wn @ z + bn) -> [d_out, i_own]
            ps_h = psumA.tile([D, SH], f32, tag="actps")
            nc.tensor.matmul(out=ps_h[:], lhsT=WnT[l][:], rhs=z_bf[:],
                             start=True, stop=True)
            hT = P.tile([D, SH], f32, tag=f"hT{l}")
            nc.scalar.activation(out=hT[:], in_=ps_h[:], func=RELU, bias=bn[l][:])
            return hT

        # ---------------- layer 0 ----------------
        if mode == "edges":
            ps_e = psumA.tile([D, SH], f32, tag="actps")
            nc.tensor.matmul(out=ps_e[:], lhsT=ident[:], rhs=me[:, 0, 0, :],
                             start=True, stop=True)
            h2T = P.tile([D, SH], f32, tag="hTdbg")
            nc.scalar.copy(h2T[:], ps_e[:])
        else:
            h1T = layer(0, xT0, xownT)

        if mode == "l1":
            h2T = h1T
        elif mode == "nocc":
            h2T = layer(1, xT0, h1T)
        elif mode == "x4":
            h = layer(1, xT0, h1T)
            h = layer(0, xT0, h)
            h2T = layer(1, xT0, h)
        elif mode == "full":
            # ------------- allgather updated node features -------------
            h1T_bf = P.tile([D, SH], bf16)
            nc.vector.tensor_scalar(out=h1T_bf[:], in0=h1T[:], scalar1=0.0,
                                    scalar2=None, op0=ADD)
            ps_t = psumA.tile([SH, D], bf16, tag="actps")
            nc.tensor.transpose(ps_t[:], h1T_bf[:], ident[:])
            h1_own = P.tile([SH, D], f32)
            nc.scalar.copy(h1_own[:], ps_t[:])

            gin = dramp.tile([SH, D], f32)
            gout = dramp.tile([N, D], f32)
            nc.gpsimd.dma_start(out=gin[:], in_=h1_own[:])
            nc.gpsimd.collective_compute(
                "AllGather", mybir.AluOpType.bypass,
                replica_groups=[list(range(NC))],
                ins=[gin[:].opt()], outs=[gout[:].opt()])

            # x1T [c, j] bf16 from gathered [N, D] f32: cast + xbar-transpose
            x1b = P.tile([128, NJT, D], bf16)
            nc.gpsimd.dma_start(
                out=x1b[:], in_=gout[:].rearrange("(jt p) d -> p jt d", p=128))
            x1T = P.tile([D, NJT, 128], bf16)
            nc.sync.dma_start(out=x1T[:], in_=x1b[:], transpose=True)
            x1T = x1T[:].rearrange("d jt j -> d (jt j)")

            # ---------------- layer 1 ----------------
            h2T = layer(1, x1T, h1T)

        # ---------------- output ----------------
        h2T_bf = P.tile([D, SH], bf16)
        nc.vector.tensor_scalar(out=h2T_bf[:], in0=h2T[:], scalar1=0.0,
                                scalar2=None, op0=ADD)
        ps_o = psumA.tile([SH, D], bf16, tag="actps")
        nc.tensor.transpose(ps_o[:], h2T_bf[:], ident[:])
        h2_own = P.tile([SH, D], f32)
        nc.scalar.copy(h2_own[:], ps_o[:])
        nc.sync.dma_start(out=out_d[:], in_=h2_own[:])

    nc.compile()
    return nc


def _host_inputs(inputs):
    """Build the 8 per-core input maps from full inputs (host-side staging)."""
    import ml_dtypes

    bf = ml_dtypes.bfloat16
    adj = np.ascontiguousarray(np.asarray(inputs["adj"], np.float32))
    nodes = np.ascontiguousarray(np.asarray(inputs["nodes"], np.float32))
    edges = np.asarray(inputs["edges"], np.float32).astype(bf)
    eps = float(np.asarray(inputs["eps"], np.float32).reshape(-1)[0])
    com = {
        "xT": np.ascontiguousarray(nodes.T.astype(bf)),
        "opse": np.full((128, 1), 1.0 + eps, np.float32),
    }
    Wne = [np.asarray(inputs["Wne0"], np.float32),
           np.asarray(inputs["Wne1"], np.float32)]
    for l in range(2):
        com[f"WeT{l}"] = np.ascontiguousarray(Wne[l][:, D:D + E].T)
        com[f"WnodeT{l}"] = np.ascontiguousarray(Wne[l][:, :D].T)
        com[f"WnT{l}"] = np.ascontiguousarray(
            np.asarray(inputs[f"Wn{l}"], np.float32).T)
        com[f"bne{l}"] = np.ascontiguousarray(
            np.asarray(inputs[f"bne{l}"], np.float32).reshape(D, 1))
        com[f"bn{l}"] = np.ascontiguousarray(
            np.asarray(inputs[f"bn{l}"], np.float32).reshape(D, 1))
    maps = []
    for c in range(NC):
        sl = slice(SH * c, SH * (c + 1))
        m = dict(com)
        m["edges_sh"] = edges[sl]
        m["adjT_sh"] = np.ascontiguousarray(adj[sl].T.astype(bf))
        m["x_ownT"] = np.ascontiguousarray(nodes[sl].T)
        maps.append(m)
    return maps


def _get_runner():
    """Build (once) a cached jit(shard_map) callable over the compiled module."""
    if "runner" in _cache:
        return _cache["runner"]
    import jax
    from jax.sharding import Mesh, PartitionSpec, NamedSharding
    from jax.experimental.shard_map import shard_map
    import concourse.mybir as mybir
    from concourse import bass2jax
    from concourse.bass2jax import _bass_exec_p, partition_id_tensor

    if "nc" not in _cache:
        _cache["nc"] = _build_nc()
    nc = _cache["nc"]
    bass2jax.install_neuronx_cc_hook()

    in_names, out_names, out_avals, zero_outs = [], [], [], []
    partition_name = nc.partition_id_tensor.name if nc.partition_id_tensor else None
    for alloc in nc.m.functions[0].allocations:
        if not isinstance(alloc, mybir.MemoryLocationSet):
            continue
        name = alloc.memorylocations[0].name
        if alloc.kind == "ExternalInput":
            if name != partition_name:
                in_names.append(name)
        elif alloc.kind == "ExternalOutput":
            shape = list(alloc.tensor_shape)
            dtype = np.dtype(mybir.dt.np(alloc.dtype))
            out_avals.append(jax.core.ShapedArray(shape, dtype))
            out_names.append(name)
            zero_outs.append(np.zeros(shape, dtype))

    n_params = len(in_names)
    all_in_names = list(in_names) + list(out_names)
    if partition_name is not None:
        all_in_names.append(partition_name)

    def _body(*args):
        operands = list(args)
        if partition_name is not None:
            operands.append(partition_id_tensor())
        outs = _bass_exec_p.bind(
            *operands,
            out_avals=tuple(out_avals),
            in_names=tuple(all_in_names),
            out_names=tuple(out_names),
            lowering_input_output_aliases=(),
            sim_require_finite=True,
            sim_require_nnan=True,
            nc=nc,
        )
        return tuple(outs)

    devices = jax.devices()[:NC]
    mesh = Mesh(np.asarray(devices), ("core",))
    n_outs = len(out_names)
    fn = jax.jit(
        shard_map(_body, mesh=mesh,
                  in_specs=(PartitionSpec("core"),) * (n_params + n_outs),
                  out_specs=(PartitionSpec("core"),) * n_outs,
                  check_rep=False),
        keep_unused=True)
    sh = NamedSharding(mesh, PartitionSpec("core"))
    dev_zeros = [
        jax.device_put(np.zeros((NC * z.shape[0], *z.shape[1:]), z.dtype), sh)
        for z in zero_outs
    ]

    def run(maps):
        dev_in = []
        for nm in in_names:
            arrs = [
                jax.device_put(np.asarray(maps[c][nm]), devices[c])
                for c in range(NC)
            ]
            shp = arrs[0].shape
            glob = jax.make_array_from_single_device_arrays(
                (NC * shp[0], *shp[1:]), sh, arrs)
            dev_in.append(glob)
        outs = fn(*dev_in, *dev_zeros)
        oi = out_names.index("out")
        return np.asarray(outs[oi]).reshape(NC, SH, D).reshape(N, D)

    _cache["runner"] = run
    return run


def kernel(**inputs):
    run = _get_runner()
    maps = _host_inputs(inputs)
    return np.ascontiguousarray(run(maps).astype(np.float32))


if __name__ == "__main__":
    _build_nc()
    print("build+compile OK")



# revision 4
# speedup vs baseline: 7.0699x; 7.0699x over previous
"""DGINConv (2-layer GIN with edge features) Trainium2 kernel — sparse/packed.

Math (per layer, reference):
    hb[j,:] = Wnode @ x[j] + bne                       # [N, D] node term
    he[i,j,:] = We @ edges[i,j,:]                      # edge term
    msg[i,:] = sum_{j: adj[i,j]=1} relu(hb[j,:] + he[i,j,:])
    out = relu(Wn @ ((1+eps)*x[i] + msg[i]) + bn)

adj density is ~3%, so instead of the dense [128 own-rows x 1024 j] sweep we
pack each own row's ~31 neighbors into padded slots (host-side, from the
actual adj at runtime):
  - own rows sorted by degree (host permutation), grouped into 8 chunks of
    16 rows; chunk c padded to S_c slots/row (mult of 4).  Q = 16*sum(S_c).
  - packed edge vectors -> peT [32, Q] bf16 (zero for pad slots)
  - slot -> source-node index list (int16), PAD slots point at column 1024
    of hbT which holds -1e9, so relu(hb_pad + 0) == 0.

Device per layer:
  hbT[d, j] = Wnode @ xT + bne (PE + ACT);  pads = -1e9
  hbg[d, q] = ap_gather(hbT, idx)           (POOL custom ucode op)
  psum[d, q] = We @ peT                     (PE, K=32)
  r[d, q] = relu(psum + hbg)                (DVE custom relu(a+b) op, or
                                             POOL add + ACT relu)
  msg[d, i] = segment-sum over S_c slots    (DVE bf16 2x tree + tensor_reduce)
  h = relu(Wn @ ((1+eps)x + msg) + bn)      (PE + ACT)

Between layers: transpose h1 -> [i,d], AllGather (rows stay in per-core
sorted order; layer-2 gather indices are host-remapped to that layout).
Final output rows are un-permuted on the host.

Distribution: destination rows sharded 8 ways; nodes/weights replicated;
one AllGather of updated node features between layers.
"""

import sys

if "/opt/trn_rl_repo" not in sys.path:
    sys.path.insert(0, "/opt/trn_rl_repo")

import numpy as np

N, D, E, NC = 1024, 128, 32, 8
SH = N // NC          # 128 rows per core
NCH = 8               # chunks of sorted own-rows
CHI = SH // NCH       # 16 rows per chunk
PAD = N               # hbT column holding -1e9
HBW = N + 8           # hbT width (pad cols 1024..1032)

# Chunk slot counts (padded max degree per 16-row chunk of the degree-sorted
# rows, mult of 4, same for all cores).  Recomputed from the actual adj at
# runtime; this is the value for the reference setup_inputs() graph.
S_DEFAULT = (52, 40, 36, 36, 32, 32, 28, 28)

# Exit engine per chunk: 'D' = DVE custom relu(a+b); 'A' = PE inject + ACT relu
EXIT_ENG = "DDDDDDDD"
GATHER_SPLIT = 2      # ap_gather instructions per layer

_cache = {}
_CUSTOM = {}


def _ensure_custom_op():
    """Register RELU_ADD_REDUCE_GIN: out = relu(in0 + in1); accum = sum."""
    if "op" in _CUSTOM:
        return _CUSTOM["op"]
    import concourse.dve_ops as dve_ops
    from concourse.dve_spec import Spec, Src0, Src1, relu, lower, _has_src1
    from concourse.dve_spec import Zero
    from concourse.dve_uop import DveOpSpec
    from operator import add

    name = "RELU_ADD_REDUCE_GIN"

    def _ref(in0, in1, c0, c1, c2):
        b = dve_ops._dve_relu(in0.astype(np.float32) + in1.astype(np.float32))
        return b, b.reshape(b.shape[0], -1).sum(axis=-1, keepdims=True)

    spec = Spec(body=relu(Src0 + Src1), accum=add, accum_init=Zero,
                reference=_ref)
    row = dve_ops._CUSTOM_DVE_ROW_BASE + len(dve_ops.OPS)
    assert row < 0x20
    shas = {}
    for ver in ("v3", "v4"):
        try:
            s = DveOpSpec(name=name, opcode=row, uops=lower(spec, ver=ver),
                          rd1_en=_has_src1(spec))
            shas[ver] = s.sha(ver)
        except Exception:
            pass
    op = dve_ops.DveOp(name, spec, subdim=False, uops_sha=shas)
    dve_ops.OPS.append(op)
    dve_ops.CUSTOM_DVE_SPECS[name] = spec
    dve_ops._SUB_OPCODE_FOR_NAME[name] = row
    _CUSTOM["op"] = op
    return op


def _build_nc(mode="full", S=S_DEFAULT):
    from contextlib import ExitStack

    import concourse.mybir as mybir
    import concourse.tile as tile
    from concourse import bacc
    from concourse.masks import make_identity

    relu_add = _ensure_custom_op()

    f32 = mybir.dt.float32
    bf16 = mybir.dt.bfloat16
    i16 = mybir.dt.int16
    RELU = mybir.ActivationFunctionType.Relu
    IDENT = mybir.ActivationFunctionType.Identity
    ADD = mybir.AluOpType.add
    MULT = mybir.AluOpType.mult

    S = tuple(S)
    Q = CHI * sum(S)
    cbase = [CHI * sum(S[:c]) for c in range(NCH)]

    nc = bacc.Bacc("TRN2", target_bir_lowering=False, debug=False,
                   enable_asserts=False, num_devices=NC)

    def din(name, shape, dt=None):
        return nc.dram_tensor(name, shape, dt or f32, kind="ExternalInput").ap()

    peT_d = din("peT_sh", [32, Q], bf16)         # packed edges^T (bf16, host)
    idx1_d = din("idx1_sh", [128, Q // 16], i16)  # layer-1 gather indices
    idx2_d = din("idx2_sh", [128, Q // 16], i16)  # layer-2 (allgather layout)
    xT_d = din("xT", [D, N], bf16)                # nodes.T bf16 (host)
    xsT_d = din("xsT_sh", [D, SH])                # own nodes.T, sorted order
    opse_d = din("opse", [128, 1])                # 1+eps replicated
    WeT_d = [din(f"WeT{l}", [E, D], bf16) for l in range(2)]
    WnodeT_d = [din(f"WnodeT{l}", [D, D], bf16) for l in range(2)]
    WnT_d = [din(f"WnT{l}", [D, D], bf16) for l in range(2)]
    bne_d = [din(f"bne{l}", [D, 1]) for l in range(2)]
    bn_d = [din(f"bn{l}", [D, 1]) for l in range(2)]
    out_d = nc.dram_tensor("out", [SH, D], f32, kind="ExternalOutput").ap()

    with tile.TileContext(nc) as tc, ExitStack() as ctx:
        P = ctx.enter_context(tc.tile_pool(name="persist", bufs=1))
        dramp = ctx.enter_context(tc.tile_pool(name="dram", bufs=1, space="DRAM"))
        psumC = ctx.enter_context(tc.tile_pool(name="psumC", bufs=2, space="PSUM"))
        psumH = ctx.enter_context(tc.tile_pool(name="psumH", bufs=1, space="PSUM"))
        psumF = ctx.enter_context(tc.tile_pool(name="psumF", bufs=2, space="PSUM"))
        scrp = ctx.enter_context(tc.tile_pool(name="scr", bufs=3))

        # ---------------- constants / small inputs ----------------
        ident = P.tile([128, 128], bf16)
        make_identity(nc, ident[:])

        opse = P.tile([128, 1], f32)
        nc.sync.dma_start(out=opse[:], in_=opse_d[:])
        xsT = P.tile([D, SH], f32)
        nc.sync.dma_start(out=xsT[:], in_=xsT_d[:])

        WeT, WnodeT, WnT, bne, bn = [], [], [], [], []
        for l in range(2):
            w = P.tile([E, D], bf16, tag=f"WeT{l}")
            nc.sync.dma_start(out=w[:], in_=WeT_d[l][:])
            WeT.append(w)
            wn = P.tile([D, D], bf16, tag=f"WnodeT{l}")
            nc.sync.dma_start(out=wn[:], in_=WnodeT_d[l][:])
            WnodeT.append(wn)
            wo = P.tile([D, D], bf16, tag=f"WnT{l}")
            nc.sync.dma_start(out=wo[:], in_=WnT_d[l][:])
            WnT.append(wo)
            b0 = P.tile([D, 1], f32, tag=f"bne{l}")
            nc.sync.dma_start(out=b0[:], in_=bne_d[l][:])
            bne.append(b0)
            b1 = P.tile([D, 1], f32, tag=f"bn{l}")
            nc.sync.dma_start(out=b1[:], in_=bn_d[l][:])
            bn.append(b1)

        peT = P.tile([32, Q], bf16)
        nc.sync.dma_start(out=peT[:], in_=peT_d[:])
        idx1 = P.tile([128, Q // 16], i16)
        nc.sync.dma_start(out=idx1[:], in_=idx1_d[:])
        idx2 = P.tile([128, Q // 16], i16)
        nc.sync.dma_start(out=idx2[:], in_=idx2_d[:])
        xT0 = P.tile([D, N], bf16)
        nc.sync.dma_start(out=xT0[:], in_=xT_d[:])

        # hbT: [d, j] f32 with -1e9 pad columns; shared by both layers
        hbT = P.tile([D, HBW], f32)
        nc.gpsimd.memset(hbT[:, N:HBW], -1e9)

        dve_scrap = P.tile([128, 1], f32)

        def layer(l, xT_l, xsT_l, idx):
            # ---- hb = Wnode @ x + bne ----
            psH = psumH.tile([D, N], f32, tag="hb")
            for h in range(2):
                nc.tensor.matmul(out=psH[:, 512 * h:512 * (h + 1)],
                                 lhsT=WnodeT[l][:],
                                 rhs=xT_l[:, 512 * h:512 * (h + 1)],
                                 start=True, stop=True)
            nc.scalar.activation(out=hbT[:, 0:N], in_=psH[:], func=IDENT,
                                 bias=bne[l][:])

            # ---- gather hb columns per packed slot (POOL) ----
            hbg = P.tile([D, Q], f32, tag=f"hbg{l}")
            splits = []
            per = (NCH + GATHER_SPLIT - 1) // GATHER_SPLIT
            for g in range(0, NCH, per):
                lo = cbase[g]
                hi = cbase[g + per] if g + per < NCH else Q
                splits.append((lo, hi))
            for lo, hi in splits:
                nc.gpsimd.ap_gather(
                    out_ap=hbg[:, lo:hi], in_ap=hbT[:],
                    idxs_ap=idx[:, lo // 16:hi // 16],
                    channels=128, num_elems=HBW, d=1, num_idxs=hi - lo)

            # ---- per chunk: edge matmul, relu(hb+he) exit, fold ----
            msg = P.tile([D, SH], f32, tag=f"msg{l}")
            for c in range(NCH):
                W = CHI * S[c]
                ps = psumC.tile([128, W], f32, tag="chunk")
                for s0 in range(0, W, 512):
                    s1 = min(s0 + 512, W)
                    nc.tensor.matmul(out=ps[:, s0:s1], lhsT=WeT[l][:],
                                     rhs=peT[:, cbase[c] + s0:cbase[c] + s1],
                                     start=True, stop=True)
                r = scrp.tile([128, CHI, S[c]], bf16, tag=f"r{S[c]}")
                r2 = r[:].rearrange("p a b -> p (a b)")
                if EXIT_ENG[c] == "D":
                    nc.vector._custom_dve(
                        relu_add, out=r2, in0=ps[:],
                        in1=hbg[:, cbase[c]:cbase[c] + W],
                        accum_out=dve_scrap[:])
                else:
                    nc.scalar.activation(out=r2, in_=ps[:], func=RELU)
                # fold: S -> S/2 (POOL) -> S/4 (DVE bf16 2x) -> reduce (DVE)
                h1 = S[c] // 2
                t1 = scrp.tile([128, CHI, h1], bf16, tag=f"t1{S[c]}")
                nc.gpsimd.tensor_tensor(out=t1[:], in0=r[:, :, 0:h1],
                                        in1=r[:, :, h1:S[c]], op=ADD)
                if h1 % 2 == 0:
                    h2 = h1 // 2
                    t2 = scrp.tile([128, CHI, h2], bf16, tag=f"t2{S[c]}")
                    nc.vector.tensor_tensor(out=t2[:], in0=t1[:, :, 0:h2],
                                            in1=t1[:, :, h2:h1], op=ADD)
                else:
                    t2, h2 = t1, h1
                nc.vector.tensor_reduce(
                    out=msg[:, CHI * c:CHI * (c + 1)], in_=t2[:],
                    axis=mybir.AxisListType.X, op=ADD)

            # ---- h = relu(Wn @ ((1+eps)x + msg) + bn) ----
            z_bf = P.tile([D, SH], bf16, tag=f"zbf{l}")
            nc.vector.scalar_tensor_tensor(out=z_bf[:], in0=xsT_l[:],
                                           scalar=opse[:], in1=msg[:],
                                           op0=MULT, op1=ADD)
            ps_h = psumF.tile([D, SH], f32, tag="fin")
            nc.tensor.matmul(out=ps_h[:], lhsT=WnT[l][:], rhs=z_bf[:],
                             start=True, stop=True)
            hT = P.tile([D, SH], f32, tag=f"hT{l}")
            nc.scalar.activation(out=hT[:], in_=ps_h[:], func=RELU,
                                 bias=bn[l][:])
            return hT

        # ---------------- layer 0 ----------------
        h1T = layer(0, xT0, xsT, idx1)

        if mode == "l1":
            h2T = h1T
        elif mode == "nocc":
            h2T = layer(1, xT0, h1T, idx1)
        elif mode == "x4":
            h = layer(1, xT0, h1T, idx1)
            h = layer(0, xT0, h, idx1)
            h2T = layer(1, xT0, h, idx1)
        elif mode == "full":
            # ------------- allgather updated node features -------------
            h1T_bf = P.tile([D, SH], bf16)
            nc.vector.tensor_scalar(out=h1T_bf[:], in0=h1T[:], scalar1=0.0,
                                    scalar2=None, op0=ADD)
            ps_t = psumF.tile([SH, D], bf16, tag="fin")
            nc.tensor.transpose(ps_t[:], h1T_bf[:], ident[:])
            h1_own = P.tile([SH, D], f32)
            nc.scalar.copy(h1_own[:], ps_t[:])

            gin = dramp.tile([SH, D], f32)
            gout = dramp.tile([N, D], f32)
            nc.gpsimd.dma_start(out=gin[:], in_=h1_own[:])
            nc.gpsimd.collective_compute(
                "AllGather", mybir.AluOpType.bypass,
                replica_groups=[list(range(NC))],
                ins=[gin[:].opt()], outs=[gout[:].opt()])

            # x1T [d, j] bf16 from gathered [N, D] f32: cast + xbar-transpose
            x1b = P.tile([128, N // 128, D], bf16)
            nc.gpsimd.dma_start(
                out=x1b[:], in_=gout[:].rearrange("(jt p) d -> p jt d", p=128))
            x1T = P.tile([D, N // 128, 128], bf16)
            nc.sync.dma_start(out=x1T[:], in_=x1b[:], transpose=True)
            x1T = x1T[:].rearrange("d jt j -> d (jt j)")

            # ---------------- layer 1 ----------------
            h2T = layer(1, x1T, h1T, idx2)

        # ---------------- output (rows in sorted order) ----------------
        h2T_bf = P.tile([D, SH], bf16)
        nc.vector.tensor_scalar(out=h2T_bf[:], in0=h2T[:], scalar1=0.0,
                                scalar2=None, op0=ADD)
        ps_o = psumF.tile([SH, D], bf16, tag="fin")
        nc.tensor.transpose(ps_o[:], h2T_bf[:], ident[:])
        h2_own = P.tile([SH, D], f32)
        nc.scalar.copy(h2_own[:], ps_o[:])
        nc.sync.dma_start(out=out_d[:], in_=h2_own[:])

    nc.compile()
    return nc


def _plan(adj):
    """Degree-sort rows per core, bucket into NCH chunks, pad to mult of 4."""
    deg = adj.sum(1).astype(np.int64).reshape(NC, SH)
    perms = [np.argsort(-deg[c], kind="stable") for c in range(NC)]
    S = []
    for ch in range(NCH):
        mx = max(int(deg[c][perms[c][CHI * ch:CHI * (ch + 1)]].max())
                 for c in range(NC))
        S.append(max(4, int(-(-mx // 4) * 4)))
    return perms, tuple(S)


def _wrap_idx(L):
    """ap_gather index layout: [128, Q//16], idx[p, m] = L[m*16 + p%16]."""
    w = L.reshape(-1, 16).T.astype(np.int16)          # [16, Q//16]
    return np.tile(w, (8, 1))


def _host_inputs(inputs):
    """Build the 8 per-core input maps + plan from full inputs."""
    import ml_dtypes

    bf = ml_dtypes.bfloat16
    adj = np.asarray(inputs["adj"], np.float32)
    nodes = np.asarray(inputs["nodes"], np.float32)
    edges = np.asarray(inputs["edges"], np.float32)
    eps = float(np.asarray(inputs["eps"], np.float32).reshape(-1)[0])
    perms, S = _plan(adj)
    Q = CHI * sum(S)
    cbase = [CHI * sum(S[:c]) for c in range(NCH)]

    # global position of node j in the allgathered (per-core sorted) layout
    gpos = np.empty(N, np.int64)
    for c in range(NC):
        gpos[c * SH + perms[c]] = c * SH + np.arange(SH)

    com = {
        "xT": np.ascontiguousarray(nodes.T.astype(bf)),
        "opse": np.full((128, 1), 1.0 + eps, np.float32),
    }
    Wne = [np.asarray(inputs["Wne0"], np.float32),
           np.asarray(inputs["Wne1"], np.float32)]
    for l in range(2):
        com[f"WeT{l}"] = np.ascontiguousarray(Wne[l][:, D:D + E].T.astype(bf))
        com[f"WnodeT{l}"] = np.ascontiguousarray(Wne[l][:, :D].T.astype(bf))
        com[f"WnT{l}"] = np.ascontiguousarray(
            np.asarray(inputs[f"Wn{l}"], np.float32).T.astype(bf))
        com[f"bne{l}"] = np.ascontiguousarray(
            np.asarray(inputs[f"bne{l}"], np.float32).reshape(D, 1))
        com[f"bn{l}"] = np.ascontiguousarray(
            np.asarray(inputs[f"bn{l}"], np.float32).reshape(D, 1))

    maps = []
    for c in range(NC):
        perm = perms[c]
        rows = c * SH + perm                       # global ids, sorted order
        pe = np.zeros((Q, E), np.float32)
        L1 = np.full(Q, PAD, np.int64)
        L2 = np.full(Q, PAD, np.int64)
        for p in range(SH):
            ch = p // CHI
            il = p % CHI
            base = cbase[ch] + il * S[ch]
            nbr = np.nonzero(adj[rows[p]])[0]
            k = len(nbr)
            assert k <= S[ch]
            pe[base:base + k] = edges[rows[p], nbr]
            L1[base:base + k] = nbr
            L2[base:base + k] = gpos[nbr]
        m = dict(com)
        m["peT_sh"] = np.ascontiguousarray(pe.T.astype(bf))
        m["idx1_sh"] = _wrap_idx(L1)
        m["idx2_sh"] = _wrap_idx(L2)
        m["xsT_sh"] = np.ascontiguousarray(nodes[rows].T)
        maps.append(m)
    return maps, perms, S


def _get_runner(S):
    """Build (once per S) a cached jit(shard_map) callable."""
    key = ("runner", S)
    if key in _cache:
        return _cache[key]
    import jax
    from jax.sharding import Mesh, PartitionSpec, NamedSharding
    from jax.experimental.shard_map import shard_map
    import concourse.mybir as mybir
    from concourse import bass2jax
    from concourse.bass2jax import _bass_exec_p, partition_id_tensor

    nckey = ("nc", S)
    if nckey not in _cache:
        _cache[nckey] = _build_nc("full", S)
    nc = _cache[nckey]
    bass2jax.install_neuronx_cc_hook()

    in_names, out_names, out_avals, zero_outs = [], [], [], []
    partition_name = nc.partition_id_tensor.name if nc.partition_id_tensor else None
    for alloc in nc.m.functions[0].allocations:
        if not isinstance(alloc, mybir.MemoryLocationSet):
            continue
        name = alloc.memorylocations[0].name
        if alloc.kind == "ExternalInput":
            if name != partition_name:
                in_names.append(name)
        elif alloc.kind == "ExternalOutput":
            shape = list(alloc.tensor_shape)
            dtype = np.dtype(mybir.dt.np(alloc.dtype))
            out_avals.append(jax.core.ShapedArray(shape, dtype))
            out_names.append(name)
            zero_outs.append(np.zeros(shape, dtype))

    n_params = len(in_names)
    all_in_names = list(in_names) + list(out_names)
    if partition_name is not None:
        all_in_names.append(partition_name)

    def _body(*args):
        operands = list(args)
        if partition_name is not None:
            operands.append(partition_id_tensor())
        outs = _bass_exec_p.bind(
            *operands,
            out_avals=tuple(out_avals),
            in_names=tuple(all_in_names),
            out_names=tuple(out_names),
            lowering_input_output_aliases=(),
            sim_require_finite=True,
            sim_require_nnan=True,
            nc=nc,
        )
        return tuple(outs)

    devices = jax.devices()[:NC]
    mesh = Mesh(np.asarray(devices), ("core",))
    n_outs = len(out_names)
    fn = jax.jit(
        shard_map(_body, mesh=mesh,
                  in_specs=(PartitionSpec("core"),) * (n_params + n_outs),
                  out_specs=(PartitionSpec("core"),) * n_outs,
                  check_rep=False),
        keep_unused=True)
    sh = NamedSharding(mesh, PartitionSpec("core"))
    dev_zeros = [
        jax.device_put(np.zeros((NC * z.shape[0], *z.shape[1:]), z.dtype), sh)
        for z in zero_outs
    ]

    def run(maps):
        dev_in = []
        for nm in in_names:
            arrs = [
                jax.device_put(np.asarray(maps[c][nm]), devices[c])
                for c in range(NC)
            ]
            shp = arrs[0].shape
            glob = jax.make_array_from_single_device_arrays(
                (NC * shp[0], *shp[1:]), sh, arrs)
            dev_in.append(glob)
        outs = fn(*dev_in, *dev_zeros)
        oi = out_names.index("out")
        return np.asarray(outs[oi]).reshape(NC, SH, D)

    _cache[key] = run
    return run


def kernel(**inputs):
    maps, perms, S = _host_inputs(inputs)
    run = _get_runner(S)
    raw = run(maps)                                # [NC, SH, D], sorted rows
    out = np.empty((N, D), np.float32)
    for c in range(NC):
        out[c * SH + perms[c]] = raw[c]
    return np.ascontiguousarray(out.astype(np.float32))


if __name__ == "__main__":
    _build_nc("nocc")
    print("build+compile OK")


# revision 18
# speedup vs baseline: 10.5525x; 1.4926x over previous
"""DGINConv (2-layer GIN with edge features) Trainium2 kernel — sparse/packed.

Math (per layer, reference):
    hb[j,:] = Wnode @ x[j] + bne                       # [N, D] node term
    he[i,j,:] = We @ edges[i,j,:]                      # edge term
    msg[i,:] = sum_{j: adj[i,j]=1} relu(hb[j,:] + he[i,j,:])
    out = relu(Wn @ ((1+eps)*x[i] + msg[i]) + bn)

adj density is ~3%, so instead of the dense [128 own-rows x 1024 j] sweep we
pack each own row's ~31 neighbors into padded slots (host-side, from the
actual adj at runtime):
  - own rows sorted by degree (host permutation), grouped into 8 chunks of
    16 rows; chunk c padded to S_c slots/row (mult of 4).  Q = 16*sum(S_c).
  - packed edge vectors -> peT [32, Q] bf16 (zero for pad slots)
  - slot -> source-node index list (int16), PAD slots point at column 1024
    of hbT which holds -1e9, so relu(hb_pad + 0) == 0.

Device per layer:
  hbT[d, j] = Wnode @ xT + bne (PE + ACT);  pads = -1e9
  hbg[d, q] = ap_gather(hbT, idx)           (POOL custom ucode op)
  psum[d, q] = We @ peT                     (PE, K=32)
  r[d, q] = relu(psum + hbg)                (DVE custom relu(a+b) op, or
                                             POOL add + ACT relu)
  msg[d, i] = segment-sum over S_c slots    (DVE bf16 2x tree + tensor_reduce)
  h = relu(Wn @ ((1+eps)x + msg) + bn)      (PE + ACT)

Between layers: transpose h1 -> [i,d], AllGather (rows stay in per-core
sorted order; layer-2 gather indices are host-remapped to that layout).
Final output rows are un-permuted on the host.

Distribution: destination rows sharded 8 ways; nodes/weights replicated;
one AllGather of updated node features between layers.
"""

import sys

if "/opt/trn_rl_repo" not in sys.path:
    sys.path.insert(0, "/opt/trn_rl_repo")

import numpy as np

N, D, E, NC = 1024, 128, 32, 8
SH = N // NC          # 128 rows per core
NCH = 8               # chunks of sorted own-rows
CHI = SH // NCH       # 16 rows per chunk
PAD = N               # hbT column holding -1e9
HBW = N + 8           # hbT width (pad cols 1024..1032)

# Chunk slot counts (padded max degree per 16-row chunk of the degree-sorted
# rows, mult of 4, same for all cores).  Recomputed from the actual adj at
# runtime; this is the value for the reference setup_inputs() graph.
S_DEFAULT = (52, 40, 36, 36, 32, 32, 28, 28)

# Exit engine per chunk: 'D' = DVE custom relu(a+b); 'A' = PE inject + ACT relu
EXIT_ENG = "DADADADA"
GATHER_SPLIT = 4      # ap_gather instructions per layer

_cache = {}
_CUSTOM = {}


def _ensure_custom_op():
    """Register RELU_ADD_REDUCE_GIN: out = relu(in0 + in1); accum = sum."""
    if "op" in _CUSTOM:
        return _CUSTOM["op"]
    import concourse.dve_ops as dve_ops
    from concourse.dve_spec import Spec, Src0, Src1, relu, lower, _has_src1
    from concourse.dve_spec import Zero
    from concourse.dve_uop import DveOpSpec
    from operator import add

    name = "RELU_ADD_REDUCE_GIN"

    def _ref(in0, in1, c0, c1, c2):
        b = dve_ops._dve_relu(in0.astype(np.float32) + in1.astype(np.float32))
        return b, b.reshape(b.shape[0], -1).sum(axis=-1, keepdims=True)

    spec = Spec(body=relu(Src0 + Src1), accum=add, accum_init=Zero,
                reference=_ref)
    row = dve_ops._CUSTOM_DVE_ROW_BASE + len(dve_ops.OPS)
    assert row < 0x20
    shas = {}
    for ver in ("v3", "v4"):
        try:
            s = DveOpSpec(name=name, opcode=row, uops=lower(spec, ver=ver),
                          rd1_en=_has_src1(spec))
            shas[ver] = s.sha(ver)
        except Exception:
            pass
    op = dve_ops.DveOp(name, spec, subdim=False, uops_sha=shas)
    dve_ops.OPS.append(op)
    dve_ops.CUSTOM_DVE_SPECS[name] = spec
    dve_ops._SUB_OPCODE_FOR_NAME[name] = row
    _CUSTOM["op"] = op
    return op


def _build_nc(mode="full", S=S_DEFAULT):
    from contextlib import ExitStack

    import concourse.mybir as mybir
    import concourse.tile as tile
    from concourse import bacc
    from concourse.masks import make_identity

    relu_add = _ensure_custom_op()

    f32 = mybir.dt.float32
    f32r = mybir.dt.float32r
    bf16 = mybir.dt.bfloat16
    i16 = mybir.dt.int16
    RELU = mybir.ActivationFunctionType.Relu
    IDENT = mybir.ActivationFunctionType.Identity
    ADD = mybir.AluOpType.add
    MULT = mybir.AluOpType.mult

    S = tuple(S)
    Q = CHI * sum(S)
    cbase = [CHI * sum(S[:c]) for c in range(NCH)]

    nc = bacc.Bacc("TRN2", target_bir_lowering=False, debug=False,
                   enable_asserts=False, num_devices=NC)

    def din(name, shape, dt=None):
        return nc.dram_tensor(name, shape, dt or f32, kind="ExternalInput").ap()

    # batched inputs (few wide DMAs, spread over queues)
    peT_d = din("peT_sh", [32, Q], bf16)          # packed edges^T
    idx_d = din("idx_sh", [128, 2 * (Q // 16)], i16)  # layer-1 ++ layer-2 idx
    xT_d = din("xT", [D, N], bf16)                # nodes.T bf16
    xsT_d = din("xsT_sh", [D, SH])                # own nodes.T, sorted order
    Wb_d = din("Wb", [D, 4 * D], bf16)            # WnodeT0|WnT0|WnodeT1|WnT1
    WeT2_d = din("WeT2", [E, 2 * D], bf16)        # WeT0|WeT1
    bias_d = din("bias", [D, 5])                  # bne0|bn0|bne1|bn1|opse
    out_d = nc.dram_tensor("out", [SH, D], f32, kind="ExternalOutput").ap()

    with tile.TileContext(nc) as tc, ExitStack() as ctx:
        P = ctx.enter_context(tc.tile_pool(name="persist", bufs=1))
        dramp = ctx.enter_context(tc.tile_pool(name="dram", bufs=1, space="DRAM"))
        psumC = ctx.enter_context(tc.tile_pool(name="psumC", bufs=2, space="PSUM"))
        psumH = ctx.enter_context(tc.tile_pool(name="psumH", bufs=1, space="PSUM"))
        psumF = ctx.enter_context(tc.tile_pool(name="psumF", bufs=2, space="PSUM"))
        scrp = ctx.enter_context(tc.tile_pool(name="scr", bufs=3))

        # ---------------- inputs (4 queues in parallel) ----------------
        xT0 = P.tile([D, N], bf16)
        nc.sync.dma_start(out=xT0[:], in_=xT_d[:])
        peT = P.tile([32, Q], bf16)
        nc.sync.dma_start(out=peT[:], in_=peT_d[:])
        idx12 = P.tile([128, 2 * (Q // 16)], i16)
        nc.gpsimd.dma_start(out=idx12[:], in_=idx_d[:])
        Wb = P.tile([D, 4 * D], bf16)
        nc.scalar.dma_start(out=Wb[:], in_=Wb_d[:])
        WeT2 = P.tile([E, 2 * D], bf16)
        nc.scalar.dma_start(out=WeT2[:], in_=WeT2_d[:])
        bias = P.tile([D, 5], f32)
        nc.scalar.dma_start(out=bias[:], in_=bias_d[:])
        xsT = P.tile([D, SH], f32)
        nc.sync.dma_start(out=xsT[:], in_=xsT_d[:])

        # constants
        ident = P.tile([128, 128], bf16)
        make_identity(nc, ident[:])

        # hbT: [d, j] f32 with -1e9 pad columns; shared by both layers
        hbT = P.tile([D, HBW], f32)
        nc.gpsimd.memset(hbT[:, N:HBW], -1e9)

        dve_scrap = P.tile([128, 1], f32)

        def layer(l, xT_l, xsT_l, idx_half):
            WnodeT = Wb[:, 2 * D * l:2 * D * l + D]
            WnT = Wb[:, 2 * D * l + D:2 * D * l + 2 * D]
            WeT = WeT2[:, D * l:D * (l + 1)]
            bne = bias[:, 2 * l:2 * l + 1]
            bn = bias[:, 2 * l + 1:2 * l + 2]
            iof = idx_half * (Q // 16)

            # ---- hb = Wnode @ x + bne ----
            psH = psumH.tile([D, N], f32, tag="hb")
            for h in range(2):
                nc.tensor.matmul(out=psH[:, 512 * h:512 * (h + 1)],
                                 lhsT=WnodeT,
                                 rhs=xT_l(h),
                                 start=True, stop=True)
            nc.scalar.activation(out=hbT[:, 0:N], in_=psH[:], func=IDENT,
                                 bias=bne)

            # ---- gather hb columns per packed slot (POOL) ----
            hbg = P.tile([D, Q], f32, tag=f"hbg{l}")
            per = (NCH + GATHER_SPLIT - 1) // GATHER_SPLIT
            for g in range(0, NCH, per):
                lo = cbase[g]
                hi = cbase[g + per] if g + per < NCH else Q
                nc.gpsimd.ap_gather(
                    out_ap=hbg[:, lo:hi], in_ap=hbT[:],
                    idxs_ap=idx12[:, iof + lo // 16:iof + hi // 16],
                    channels=128, num_elems=HBW, d=1, num_idxs=hi - lo)

            # ---- per chunk: edge matmul, relu(hb+he) exit, fold ----
            msg = P.tile([D, SH], f32, tag=f"msg{l}")
            for c in range(NCH):
                W = CHI * S[c]
                act = EXIT_ENG[c] == "A"
                ps = psumC.tile([128, W], f32, tag="chunk")
                if act:
                    hgb = scrp.tile([128, W], bf16, tag=f"hgb{S[c]}")
                    nc.scalar.activation(
                        out=hgb[:], in_=hbg[:, cbase[c]:cbase[c] + W],
                        func=IDENT)
                for s0 in range(0, W, 512):
                    s1 = min(s0 + 512, W)
                    nc.tensor.matmul(out=ps[:, s0:s1], lhsT=WeT,
                                     rhs=peT[:, cbase[c] + s0:cbase[c] + s1],
                                     start=True, stop=not act)
                    if act:
                        nc.tensor.matmul(
                            out=ps[:, s0:s1], lhsT=ident[:],
                            rhs=hgb[:, s0:s1],
                            start=False, stop=True)
                r = scrp.tile([128, CHI, S[c]], bf16, tag=f"r{S[c]}")
                r2 = r[:].rearrange("p a b -> p (a b)")
                if act:
                    nc.scalar.activation(out=r2, in_=ps[:], func=RELU)
                else:
                    nc.vector._custom_dve(
                        relu_add, out=r2, in0=ps[:],
                        in1=hbg[:, cbase[c]:cbase[c] + W],
                        accum_out=dve_scrap[:])
                # fold: S -> S/2 -> S/4 (DVE bf16 2x) -> reduce (DVE)
                h1 = S[c] // 2
                t1 = scrp.tile([128, CHI, h1], bf16, tag=f"t1{S[c]}")
                nc.vector.tensor_tensor(out=t1[:], in0=r[:, :, 0:h1],
                                        in1=r[:, :, h1:S[c]], op=ADD)
                if h1 % 2 == 0:
                    h2 = h1 // 2
                    t2 = scrp.tile([128, CHI, h2], bf16, tag=f"t2{S[c]}")
                    nc.vector.tensor_tensor(out=t2[:], in0=t1[:, :, 0:h2],
                                            in1=t1[:, :, h2:h1], op=ADD)
                else:
                    t2, h2 = t1, h1
                nc.vector.tensor_reduce(
                    out=msg[:, CHI * c:CHI * (c + 1)], in_=t2[:],
                    axis=mybir.AxisListType.X, op=ADD)

            # ---- h = relu(Wn @ ((1+eps)x + msg) + bn) ----
            z_bf = P.tile([D, SH], bf16, tag=f"zbf{l}")
            nc.vector.scalar_tensor_tensor(out=z_bf[:], in0=xsT_l[:],
                                           scalar=bias[:, 4:5], in1=msg[:],
                                           op0=MULT, op1=ADD)
            ps_h = psumF.tile([D, SH], f32, tag="fin")
            nc.tensor.matmul(out=ps_h[:], lhsT=WnT, rhs=z_bf[:],
                             start=True, stop=True)
            hT = P.tile([D, SH], f32, tag=f"hT{l}")
            nc.scalar.activation(out=hT[:], in_=ps_h[:], func=RELU,
                                 bias=bn)
            return hT

        # ---------------- layer 0 ----------------
        def x0(h):
            return xT0[:, 512 * h:512 * (h + 1)]

        h1T = layer(0, x0, xsT, 0)

        if mode == "l1":
            h2T = h1T
        elif mode == "nocc":
            h2T = layer(1, x0, h1T, 0)
        elif mode == "x4":
            h = layer(1, x0, h1T, 0)
            h = layer(0, x0, h, 0)
            h2T = layer(1, x0, h, 0)
        elif mode == "full":
            # ------------- allgather updated node features -------------
            h1T_bf = P.tile([D, SH], bf16)
            nc.vector.tensor_scalar(out=h1T_bf[:], in0=h1T[:], scalar1=0.0,
                                    scalar2=None, op0=ADD)
            ps_t = psumF.tile([SH, D], bf16, tag="fin")
            nc.tensor.transpose(ps_t[:], h1T_bf[:], ident[:])
            h1_own = P.tile([SH, D], f32)
            nc.scalar.copy(h1_own[:], ps_t[:])

            gin = dramp.tile([SH, D], f32)
            gout = dramp.tile([N, D], f32)
            nc.gpsimd.dma_start(out=gin[:], in_=h1_own[:])
            nc.gpsimd.collective_compute(
                "AllGather", mybir.AluOpType.bypass,
                replica_groups=[list(range(NC))],
                ins=[gin[:].opt()], outs=[gout[:].opt()])

            # x1T [d, j] bf16 from gathered [N, D] f32: cast + xbar-transpose
            x1b = P.tile([128, N // 128, D], bf16)
            nc.gpsimd.dma_start(
                out=x1b[:], in_=gout[:].rearrange("(jt p) d -> p jt d", p=128))
            x1T = P.tile([D, N // 128, 128], bf16)
            nc.sync.dma_start(out=x1T[:], in_=x1b[:], transpose=True)

            def x1(h):
                return x1T[:, 4 * h:4 * (h + 1), :]

            # ---------------- layer 1 ----------------
            h2T = layer(1, x1, h1T, 1)

        # ---------------- output (rows in sorted order) ----------------
        h2T_bf = P.tile([D, SH], bf16)
        nc.vector.tensor_scalar(out=h2T_bf[:], in0=h2T[:], scalar1=0.0,
                                scalar2=None, op0=ADD)
        ps_o = psumF.tile([SH, D], bf16, tag="fin")
        nc.tensor.transpose(ps_o[:], h2T_bf[:], ident[:])
        h2_own = P.tile([SH, D], f32)
        nc.scalar.copy(h2_own[:], ps_o[:])
        nc.sync.dma_start(out=out_d[:], in_=h2_own[:])

    nc.compile()
    return nc


def _plan(adj):
    """Degree-sort rows per core, bucket into NCH chunks, pad to mult of 4."""
    deg = adj.sum(1).astype(np.int64).reshape(NC, SH)
    perms = [np.argsort(-deg[c], kind="stable") for c in range(NC)]
    S = []
    for ch in range(NCH):
        mx = max(int(deg[c][perms[c][CHI * ch:CHI * (ch + 1)]].max())
                 for c in range(NC))
        S.append(max(4, int(-(-mx // 4) * 4)))
    return perms, tuple(S)


def _wrap_idx(L):
    """ap_gather index layout: [128, Q//16], idx[p, m] = L[m*16 + p%16]."""
    w = L.reshape(-1, 16).T.astype(np.int16)          # [16, Q//16]
    return np.tile(w, (8, 1))


def _host_inputs(inputs):
    """Build the 8 per-core input maps + plan from full inputs."""
    import ml_dtypes

    bf = ml_dtypes.bfloat16
    adj = np.asarray(inputs["adj"], np.float32)
    nodes = np.asarray(inputs["nodes"], np.float32)
    edges = np.asarray(inputs["edges"], np.float32)
    eps = float(np.asarray(inputs["eps"], np.float32).reshape(-1)[0])
    perms, S = _plan(adj)
    Q = CHI * sum(S)
    cbase = [CHI * sum(S[:c]) for c in range(NCH)]

    # global position of node j in the allgathered (per-core sorted) layout
    gpos = np.empty(N, np.int64)
    for c in range(NC):
        gpos[c * SH + perms[c]] = c * SH + np.arange(SH)

    Wne = [np.asarray(inputs["Wne0"], np.float32),
           np.asarray(inputs["Wne1"], np.float32)]
    Wb = np.concatenate(
        [np.concatenate(
            [Wne[l][:, :D].T,
             np.asarray(inputs[f"Wn{l}"], np.float32).T], axis=1)
         for l in range(2)], axis=1)
    WeT2 = np.concatenate([Wne[0][:, D:D + E].T, Wne[1][:, D:D + E].T], axis=1)
    bias = np.stack(
        [np.asarray(inputs["bne0"], np.float32),
         np.asarray(inputs["bn0"], np.float32),
         np.asarray(inputs["bne1"], np.float32),
         np.asarray(inputs["bn1"], np.float32),
         np.full(D, 1.0 + eps, np.float32)], axis=1)
    com = {
        "xT": np.ascontiguousarray(nodes.T.astype(bf)),
        "Wb": np.ascontiguousarray(Wb.astype(bf)),
        "WeT2": np.ascontiguousarray(WeT2.astype(bf)),
        "bias": np.ascontiguousarray(bias),
    }

    maps = []
    for c in range(NC):
        perm = perms[c]
        rows = c * SH + perm                       # global ids, sorted order
        pe = np.zeros((Q, E), np.float32)
        L1 = np.full(Q, PAD, np.int64)
        L2 = np.full(Q, PAD, np.int64)
        for p in range(SH):
            ch = p // CHI
            il = p % CHI
            base = cbase[ch] + il * S[ch]
            nbr = np.nonzero(adj[rows[p]])[0]
            k = len(nbr)
            assert k <= S[ch]
            pe[base:base + k] = edges[rows[p], nbr]
            L1[base:base + k] = nbr
            L2[base:base + k] = gpos[nbr]
        m = dict(com)
        m["peT_sh"] = np.ascontiguousarray(pe.T.astype(bf))
        m["idx_sh"] = np.ascontiguousarray(
            np.concatenate([_wrap_idx(L1), _wrap_idx(L2)], axis=1))
        m["xsT_sh"] = np.ascontiguousarray(nodes[rows].T)
        maps.append(m)
    return maps, perms, S


def _get_runner(S):
    """Build (once per S) a cached jit(shard_map) callable."""
    key = ("runner", S)
    if key in _cache:
        return _cache[key]
    import jax
    from jax.sharding import Mesh, PartitionSpec, NamedSharding
    from jax.experimental.shard_map import shard_map
    import concourse.mybir as mybir
    from concourse import bass2jax
    from concourse.bass2jax import _bass_exec_p, partition_id_tensor

    nckey = ("nc", S)
    if nckey not in _cache:
        _cache[nckey] = _build_nc("full", S)
    nc = _cache[nckey]
    bass2jax.install_neuronx_cc_hook()

    in_names, out_names, out_avals, zero_outs = [], [], [], []
    partition_name = nc.partition_id_tensor.name if nc.partition_id_tensor else None
    for alloc in nc.m.functions[0].allocations:
        if not isinstance(alloc, mybir.MemoryLocationSet):
            continue
        name = alloc.memorylocations[0].name
        if alloc.kind == "ExternalInput":
            if name != partition_name:
                in_names.append(name)
        elif alloc.kind == "ExternalOutput":
            shape = list(alloc.tensor_shape)
            dtype = np.dtype(mybir.dt.np(alloc.dtype))
            out_avals.append(jax.core.ShapedArray(shape, dtype))
            out_names.append(name)
            zero_outs.append(np.zeros(shape, dtype))

    n_params = len(in_names)
    all_in_names = list(in_names) + list(out_names)
    if partition_name is not None:
        all_in_names.append(partition_name)

    def _body(*args):
        operands = list(args)
        if partition_name is not None:
            operands.append(partition_id_tensor())
        outs = _bass_exec_p.bind(
            *operands,
            out_avals=tuple(out_avals),
            in_names=tuple(all_in_names),
            out_names=tuple(out_names),
            lowering_input_output_aliases=(),
            sim_require_finite=True,
            sim_require_nnan=True,
            nc=nc,
        )
        return tuple(outs)

    devices = jax.devices()[:NC]
    mesh = Mesh(np.asarray(devices), ("core",))
    n_outs = len(out_names)
    fn = jax.jit(
        shard_map(_body, mesh=mesh,
                  in_specs=(PartitionSpec("core"),) * (n_params + n_outs),
                  out_specs=(PartitionSpec("core"),) * n_outs,
                  check_rep=False),
        keep_unused=True)
    sh = NamedSharding(mesh, PartitionSpec("core"))
    dev_zeros = [
        jax.device_put(np.zeros((NC * z.shape[0], *z.shape[1:]), z.dtype), sh)
        for z in zero_outs
    ]

    def run(maps):
        dev_in = []
        for nm in in_names:
            arrs = [
                jax.device_put(np.asarray(maps[c][nm]), devices[c])
                for c in range(NC)
            ]
            shp = arrs[0].shape
            glob = jax.make_array_from_single_device_arrays(
                (NC * shp[0], *shp[1:]), sh, arrs)
            dev_in.append(glob)
        outs = fn(*dev_in, *dev_zeros)
        oi = out_names.index("out")
        return np.asarray(outs[oi]).reshape(NC, SH, D)

    _cache[key] = run
    return run


def kernel(**inputs):
    maps, perms, S = _host_inputs(inputs)
    run = _get_runner(S)
    raw = run(maps)                                # [NC, SH, D], sorted rows
    out = np.empty((N, D), np.float32)
    for c in range(NC):
        out[c * SH + perms[c]] = raw[c]
    return np.ascontiguousarray(out.astype(np.float32))


if __name__ == "__main__":
    _build_nc("nocc")
    print("build+compile OK")
